# revision 22
# baseline (speedup 1.0000x reference)
"""Trainium2 Bass kernel for a 4-layer LSTM decoder step with Bahdanau attention.

v2: bf16 streaming, gates-on-free-dim LSTM, PE-based context accumulation,
coalesced [128, N] DMAs (bulk on SWDGE, latency-critical on SP HWDGE),
bf16 collectives, proj ctx-half interleaved into collective gaps.

Math (B=128 batch, S=128 enc positions, H=A=E_enc=1024, emb=64, V=32000, NL=4):
  x   = E[tokens]
  o1  = LSTM_f([x, context], hidden0, cell0)
  ad  = o1 @ Wad.T + bad ; w = ad @ Wae ; cdot = ad . bae
  scores[s,b] = enc[s,b,:].w[b,:] + cdot[b] ; alpha = softmax_s
  ctx = sum_s alpha[s,b] * enc[s,b,:]   (accumulated as diag(alpha_s) @ enc_s)
  h   = LSTM_l0([o1, ctx]) -> LSTM_r1(h) -> LSTM_r2(h)
  out = [h, ctx] @ Wout.T + bout                               # [128, 32000]

Distribution over 8 NeuronCores:
  - LSTM: gate rows sharded 8-way (each core computes a 128-wide hidden
    shard); full h re-assembled with a bf16 AllGather after every layer.
  - Attention: sharded over encoder positions s (16 per core); partial
    exp-weighted context + sum(exp) combined with one bf16 AllReduce.
  - Output projection: vocab-sharded (4000 rows of Wout per core, padded
    to 4096); shards concatenated on the host.
"""
import os
import sys

sys.path.insert(0, "/opt/trn_rl_repo")

STAGE = int(os.environ.get("KERNEL_STAGE", "4"))
NOCC = bool(int(os.environ.get("KERNEL_NOCC", "0")))  # timing-sim stand-in mode
CCF32 = bool(int(os.environ.get("KERNEL_CCF32", "0")))  # fp32 collectives fallback

import numpy as np
import ml_dtypes

from concourse import bacc, mybir, tile
from concourse.bass_utils import run_bass_kernel_spmd

F32 = mybir.dt.float32
BF16 = mybir.dt.bfloat16
ALU = mybir.AluOpType
ACT = mybir.ActivationFunctionType
NPBF = ml_dtypes.bfloat16

B = 128          # batch
S = 128          # encoder length
H = 1024         # hidden dim
NL = 4           # LSTM layers
KATT = 128       # attention projection size
E = 1024         # encoder hidden dim
NCORES = 8
HSH = H // NCORES        # 128: hidden shard per core
SSH = S // NCORES        # 16: encoder positions per core
VSH = 32000 // NCORES    # 4000: vocab shard
VPAD = 4096              # padded vocab shard (8 x 512)
NV = VPAD // 512         # 8 vocab blocks of 512
XC = 1152                # padded [emb(64) + context(1024)] input width (9 x 128)
NCH = (XC // 128 + 8, 2 * H // 128 + 8, H // 128 + 8, H // 128 + 8)  # 17,24,16,16
CCDT = F32 if CCF32 else BF16

_compiled = None


def _build():
    nc = bacc.Bacc("TRN2", target_bir_lowering=False, debug=False,
                   num_devices=NCORES)

    def din(name, shape, dt=BF16):
        return nc.dram_tensor(name, list(shape), dt, kind="ExternalInput").ap()

    xcT = din("xcT", [128, 9 * B])              # layer-f input chunks, transposed
    hTin = din("hTin", [128, NL * 8 * B])       # prev hidden chunks (l,k) transposed
    cnat = din("cnat", [128, NL * HSH], F32)    # cell shards, natural [B, l*128+j]
    wl = [din(f"wl{l}", [128, NCH[l] * 512]) for l in range(NL)]
    brow = din("brow", [1, NL * 512])      # gate bias rows per layer
    waen = din("waen", [KATT, E])               # Wae natural
    wadT = din("wadT", [128, 8 * KATT])         # Wad.T chunks
    baec = din("baec", [KATT, 1])
    badc = din("badc", [KATT, 1], F32)
    encN = din("encN", [B, SSH * E])            # enc natural [b, s*E+e]
    woutC = din("woutC", [128, NV * 8 * 512])   # ctx-half of WoutT, per (vb,kc)
    woutH = din("woutH", [128, NV * 8 * 512])   # h-half of WoutT
    boutr = din("boutr", [1, VPAD])
    identb = din("identb", [128, 128])
    out = nc.dram_tensor("out", [B, VPAD], F32, kind="ExternalOutput").ap()

    rg = [list(range(NCORES))]

    with tile.TileContext(nc) as tc:
        with tc.tile_pool(name="const", bufs=1) as const, \
             tc.tile_pool(name="wpool", bufs=1) as wpool, \
             tc.tile_pool(name="encp", bufs=1) as encp, \
             tc.tile_pool(name="woutp", bufs=1) as woutp, \
             tc.tile_pool(name="acts", bufs=1) as acts, \
             tc.tile_pool(name="gps", bufs=1, space="PSUM") as gps, \
             tc.tile_pool(name="attps", bufs=1, space="PSUM") as attps, \
             tc.tile_pool(name="bigps", bufs=1, space="PSUM") as bigps, \
             tc.tile_pool(name="trps", bufs=1, space="PSUM") as trps, \
             tc.tile_pool(name="dram", bufs=1, space="DRAM") as dram:

            # ---------------- constant / small loads (SP HWDGE ring) --------
            def cload(name, src, shape, dt):
                t = const.tile(list(shape), dt, tag=name, name=name)
                nc.sync.dma_start(t[:], src)
                return t

            ident_sb = cload("identb", identb[:], [128, 128], BF16)
            brow_sb = cload("brow", brow[:], [1, NL * 512], BF16)
            bout_sb = cload("boutr", boutr[:], [1, VPAD], BF16)
            waen_sb = cload("waen", waen[:], [KATT, E], BF16)
            wadT_sb = cload("wadT", wadT[:], [128, 8 * KATT], BF16)
            baec_sb = cload("baec", baec[:], [KATT, 1], BF16)
            badc_sb = cload("badc", badc[:], [KATT, 1], F32)
            onesr = const.tile([1, B], BF16, tag="onesr", name="onesr")
            nc.vector.memset(onesr[:], 1.0)

            # ---------------- bulk loads (SWDGE / gpsimd ring) --------------
            # Issue order == arrival order on the ring: early-needed first.
            # Interleaved with the collectives (also on gpsimd) so
            # buffer-reuse waits cannot deadlock against them.
            def bload(name, src, shape, dt=BF16, pool=None, tag=None, bufs=None):
                t = (pool or const).tile(list(shape), dt, tag=tag or name,
                                         bufs=bufs, name=name)
                nc.gpsimd.dma_start(t[:], src)
                return t

            xcT_sb = bload("xcT", xcT[:], [128, 9 * B])
            cnat_sb = bload("cnat", cnat[:], [128, NL * HSH], F32)
            hTin_sb = bload("hTin", hTin[:], [128, NL * 8 * B])
            wl_sb = [None] * NL
            for l in (0, 1):
                wl_sb[l] = wpool.tile([128, NCH[1] * 512], BF16, tag="wl",
                                      bufs=2, name=f"wl{l}")
                nc.gpsimd.dma_start(wl_sb[l][:, :NCH[l] * 512], wl[l][:])
            encN_sb = encp.tile([B, SSH * E], BF16, tag="encN", name="encN_sb")
            half = SSH * E // 2
            nc.gpsimd.dma_start(encN_sb[:, 0:half], encN[:, 0:half])
            nc.gpsimd.dma_start(encN_sb[:, half:], encN[:, half:])
            wC_sb = [None] * NV
            wH_sb = [None] * NV

            def load_wC(vb):
                wC_sb[vb] = woutp.tile([128, 8 * 512], BF16, tag="wC", bufs=2,
                                       name="wC")
                nc.gpsimd.dma_start(wC_sb[vb][:],
                                    woutC[:, vb * 4096:(vb + 1) * 4096])

            def load_wH(vb):
                wH_sb[vb] = woutp.tile([128, 8 * 512], BF16, tag="wH", bufs=5,
                                       name="wH")
                nc.gpsimd.dma_start(wH_sb[vb][:],
                                    woutH[:, vb * 4096:(vb + 1) * 4096])

            # ---------------- helpers ---------------------------------------
            def allgather(src_sb, name):
                """src_sb [128, B] bf16 -> [128, 8*B] sbuf tile of full hT."""
                cc_in = dram.tile([HSH, B], CCDT, tag=f"agi_{name}")
                cc_out = dram.tile([H, B], CCDT, tag=f"ago_{name}")
                nc.gpsimd.dma_start(cc_in[:], src_sb[:])
                if NOCC:
                    nc.gpsimd.dma_start(cc_out[0:HSH, :], cc_in[:])
                else:
                    nc.gpsimd.collective_compute(
                        "AllGather", ALU.bypass, replica_groups=rg,
                        ins=[cc_in[:].opt()], outs=[cc_out[:].opt()])
                t = acts.tile([128, 8 * B], BF16, tag="agh", bufs=3, name="agh")
                # partition p <- rows {p, 128+p, ...}: 8 segments per partition
                src_ap = cc_out[:].rearrange("(k p) b -> p k b", p=128)
                dst_ap = t[:].rearrange("p (k b) -> p k b", k=8)
                nc.gpsimd.dma_start(dst_ap, src_ap)
                return t

            def transpose_h(h_nat, tag="htr"):
                """h_nat [B, 128] bf16 -> [128, B] bf16 sbuf tile."""
                tp = trps.tile([128, B], BF16, tag="trp", bufs=2, name="trp")
                nc.tensor.transpose(tp[:], h_nat[:], ident_sb[:])
                t = acts.tile([128, B], BF16, tag=tag, bufs=2, name=tag)
                nc.scalar.activation(t[:], tp[:], ACT.Copy)
                return t

            def lstm_layer(l, in_chunks):
                """in_chunks: list of ([128, B] tile, free-offset) bf16 chunk
                sources for the layer input (transposed). Issues the gate
                matmuls; returns (h_nat, hT) tiles."""
                gps_t = gps.tile([B, 512], F32, tag="ps512", bufs=2,
                                 name=f"gates{l}")
                nc.tensor.matmul(gps_t[:], onesr[:],
                                 brow_sb[:, l * 512:(l + 1) * 512],
                                 start=True, stop=False)
                nch = NCH[l]
                for i, (src, off) in enumerate(in_chunks):
                    nc.tensor.matmul(gps_t[:], src[:, off:off + B],
                                     wl_sb[l][:, i * 512:(i + 1) * 512],
                                     start=False, stop=(i == nch - 1))
                assert len(in_chunks) == nch
                ifgo = []
                for g, fn in enumerate((ACT.Sigmoid, ACT.Sigmoid, ACT.Tanh,
                                        ACT.Sigmoid)):
                    t = acts.tile([B, 128], F32, tag="ifgo", bufs=8, name="ifgo")
                    nc.scalar.activation(t[:], gps_t[:, g * 128:(g + 1) * 128], fn)
                    ifgo.append(t)
                t1 = acts.tile([B, 128], F32, tag="ctmp", bufs=6, name="ctmp")
                nc.vector.tensor_tensor(t1[:], ifgo[1][:],
                                        cnat_sb[:, l * HSH:(l + 1) * HSH],
                                        ALU.mult)
                t2 = acts.tile([B, 128], F32, tag="ctmp", bufs=6, name="ctmp")
                nc.vector.tensor_tensor(t2[:], ifgo[0][:], ifgo[2][:], ALU.mult)
                c2 = acts.tile([B, 128], F32, tag="ctmp", bufs=6, name="ctmp")
                nc.vector.tensor_tensor(c2[:], t1[:], t2[:], ALU.add)
                tc2 = acts.tile([B, 128], F32, tag="ctmp", bufs=6, name="ctmp")
                nc.scalar.activation(tc2[:], c2[:], ACT.Tanh)
                h = acts.tile([B, 128], BF16, tag="hnat", bufs=2, name="hnat")
                nc.vector.tensor_tensor(h[:], ifgo[3][:], tc2[:], ALU.mult)
                return h, transpose_h(h)

            def hin_chunks(l):
                return [(hTin_sb, (l * 8 + k) * B) for k in range(8)]

            # ---------------- layer f ---------------------------------------
            h1, h1T = lstm_layer(0, [(xcT_sb, k * B) for k in range(9)]
                                 + hin_chunks(0))
            o1T = allgather(h1T, "h1")
            # bulk wave 2 (post-AG1 on the gpsimd ring): layer-2 weights and
            # the first output-projection tiles
            wl_sb[2] = wpool.tile([128, NCH[1] * 512], BF16, tag="wl",
                                  bufs=2, name="wl2")
            nc.gpsimd.dma_start(wl_sb[2][:, :NCH[2] * 512], wl[2][:])
            load_wC(0)
            load_wC(1)
            load_wH(0)
            load_wH(1)
            load_wH(2)
            if STAGE == 1:
                o1f = acts.tile([128, 8 * B], F32, tag="dbg", name="o1f")
                nc.vector.tensor_copy(o1f[:], o1T[:])
                for k in range(8):
                    nc.sync.dma_start(out[:, k * 128:(k + 1) * 128],
                                      o1f[:, k * B:(k + 1) * B])

            if STAGE >= 2:
                # ---------------- attention ---------------------------------
                # adT[k, b] = Wad @ o1 + bad
                ad_ps = attps.tile([KATT, B], F32, tag="att128", bufs=2,
                                   name="ad_ps")
                for k in range(8):
                    nc.tensor.matmul(ad_ps[:], wadT_sb[:, k * KATT:(k + 1) * KATT],
                                     o1T[:, k * B:(k + 1) * B],
                                     start=(k == 0), stop=(k == 7))
                adT_sb = acts.tile([KATT, B], BF16, tag="adT", name="adT_sb")
                nc.scalar.activation(adT_sb[:], ad_ps[:], ACT.Identity,
                                     bias=badc_sb[:])
                # cdot[b] = ad . bae
                cd_ps = attps.tile([B, 1], F32, tag="att128", bufs=2, name="cd_ps")
                nc.tensor.matmul(cd_ps[:], adT_sb[:], baec_sb[:],
                                 start=True, stop=True)
                cdot_sb = acts.tile([B, 1], F32, tag="cdot", name="cdot_sb")
                nc.vector.tensor_copy(cdot_sb[:], cd_ps[:])
                # w[b, e] = ad @ Wae
                w_ps = bigps.tile([B, E], F32, tag="ps1024", bufs=1, name="w_ps")
                for half in range(2):
                    nc.tensor.matmul(w_ps[:, half * 512:(half + 1) * 512],
                                     adT_sb[:],
                                     waen_sb[:, half * 512:(half + 1) * 512],
                                     start=True, stop=True)
                w_sb = acts.tile([B, E], BF16, tag="w_sb", name="w_sb")
                nc.scalar.activation(w_sb[:], w_ps[:], ACT.Copy)
                # scores -> alpha columns; ctx accumulated via diag trick
                alpha = acts.tile([B, SSH], F32, tag="alpha", name="alpha")
                ctx_ps = bigps.tile([B, E], F32, tag="ps1024", bufs=1,
                                    name="ctx_ps")
                prodd = acts.tile([B, E], BF16, tag="prodd", bufs=1, name="prodd")
                for s in range(SSH):
                    sc = acts.tile([B, 1], F32, tag="sccol", bufs=2, name="sccol")
                    nc.vector.tensor_tensor_reduce(
                        prodd[:], encN_sb[:, s * E:(s + 1) * E], w_sb[:],
                        1.0, 0.0, ALU.mult, ALU.add, accum_out=sc[:])
                    nc.scalar.activation(alpha[:, s:s + 1], sc[:], ACT.Exp,
                                         bias=cdot_sb[:])
                    dg = acts.tile([B, B], BF16, tag="diag", bufs=2, name="diag")
                    nc.vector.tensor_tensor(dg[:], ident_sb[:],
                                            alpha[:, s:s + 1].to_broadcast([B, B]),
                                            ALU.mult)
                    for half in range(2):
                        nc.tensor.matmul(ctx_ps[:, half * 512:(half + 1) * 512],
                                         dg[:],
                                         encN_sb[:, s * E + half * 512:
                                                 s * E + (half + 1) * 512],
                                         start=(s == 0), stop=(s == SSH - 1))
                sume = acts.tile([B, 1], F32, tag="sume", name="sume")
                nc.vector.tensor_reduce(sume[:], alpha[:], mybir.AxisListType.X,
                                        ALU.add)
                # AllReduce partial [ctxU | sumexp]
                ccpad = 16 if CCDT == BF16 else 8
                ar_in = dram.tile([B, E + ccpad], CCDT, tag="ar_in")
                ar_out = dram.tile([B, E + ccpad], CCDT, tag="ar_out")
                ctxu_sb = acts.tile([B, E + ccpad], CCDT, tag="ctxu",
                                    name="ctxu_sb")
                nc.vector.tensor_copy(ctxu_sb[:, 0:E], ctx_ps[:])
                nc.vector.tensor_copy(ctxu_sb[:, E:E + ccpad],
                                      sume[:].to_broadcast([B, ccpad]))
                nc.gpsimd.dma_start(ar_in[:], ctxu_sb[:])
                if NOCC:
                    nc.gpsimd.dma_start(ar_out[:], ar_in[:])
                else:
                    nc.gpsimd.collective_compute(
                        "AllReduce", ALU.add, replica_groups=rg,
                        ins=[ar_in[:].opt()], outs=[ar_out[:].opt()])
                ctxg = acts.tile([B, E + ccpad], CCDT, tag="ctxg", name="ctxg")
                nc.gpsimd.dma_start(ctxg[:], ar_out[:])
                seg = acts.tile([B, 1], F32, tag="seg", name="seg")
                nc.vector.tensor_copy(seg[:], ctxg[:, E:E + 1])
                recip = acts.tile([B, 1], F32, tag="recip", name="recip")
                nc.vector.reciprocal(recip[:], seg[:])
                ctx_sb = acts.tile([B, E], BF16, tag="ctx_sb", name="ctx_sb")
                nc.scalar.activation(ctx_sb[:], ctxg[:, 0:E], ACT.Copy,
                                     scale=recip[:])

            if STAGE == 2:
                cf = acts.tile([B, E], F32, tag="dbg", name="cf")
                nc.vector.tensor_copy(cf[:], ctx_sb[:])
                nc.sync.dma_start(out[:, 0:E], cf[:])

            if STAGE >= 3:
                # ---------------- layer l0 ----------------------------------
                # o1-half of the gates can start before ctx is ready.
                gps_l0 = gps.tile([B, 512], F32, tag="ps512", bufs=2,
                                  name="gates1")
                nc.tensor.matmul(gps_l0[:], onesr[:], brow_sb[:, 512:1024],
                                 start=True, stop=False)
                for k in range(8):
                    nc.tensor.matmul(gps_l0[:], o1T[:, k * B:(k + 1) * B],
                                     wl_sb[1][:, k * 512:(k + 1) * 512],
                                     start=False, stop=False)
                for k in range(8):
                    nc.tensor.matmul(gps_l0[:], hTin_sb[:, (8 + k) * B:(9 + k) * B],
                                     wl_sb[1][:, (16 + k) * 512:(17 + k) * 512],
                                     start=False, stop=False)
                # ctx transposed chunks (also feed the output projection)
                ctxT = acts.tile([128, 8 * B], BF16, tag="ctxT", name="ctxT")
                for k in range(8):
                    tp = trps.tile([128, B], BF16, tag="trp", bufs=2, name="trp")
                    nc.tensor.transpose(tp[:], ctx_sb[:, k * 128:(k + 1) * 128],
                                        ident_sb[:])
                    nc.scalar.activation(ctxT[:, k * B:(k + 1) * B], tp[:],
                                         ACT.Copy)
                for k in range(8):
                    nc.tensor.matmul(gps_l0[:], ctxT[:, k * B:(k + 1) * B],
                                     wl_sb[1][:, (8 + k) * 512:(9 + k) * 512],
                                     start=False, stop=(k == 7))
                ifgo = []
                for g, fn in enumerate((ACT.Sigmoid, ACT.Sigmoid, ACT.Tanh,
                                        ACT.Sigmoid)):
                    t = acts.tile([B, 128], F32, tag="ifgo", bufs=8, name="ifgo")
                    nc.scalar.activation(t[:], gps_l0[:, g * 128:(g + 1) * 128],
                                         fn)
                    ifgo.append(t)
                t1 = acts.tile([B, 128], F32, tag="ctmp", bufs=6, name="ctmp")
                nc.vector.tensor_tensor(t1[:], ifgo[1][:],
                                        cnat_sb[:, HSH:2 * HSH], ALU.mult)
                t2 = acts.tile([B, 128], F32, tag="ctmp", bufs=6, name="ctmp")
                nc.vector.tensor_tensor(t2[:], ifgo[0][:], ifgo[2][:], ALU.mult)
                c2 = acts.tile([B, 128], F32, tag="ctmp", bufs=6, name="ctmp")
                nc.vector.tensor_tensor(c2[:], t1[:], t2[:], ALU.add)
                tc2 = acts.tile([B, 128], F32, tag="ctmp", bufs=6, name="ctmp")
                nc.scalar.activation(tc2[:], c2[:], ACT.Tanh)
                h2 = acts.tile([B, 128], BF16, tag="hnat", bufs=2, name="hnat")
                nc.vector.tensor_tensor(h2[:], ifgo[3][:], tc2[:], ALU.mult)
                h2T = transpose_h(h2)
                h2g = allgather(h2T, "h2")
                # bulk: layer-3 weights (reuses wl1's slot; l0 mms are done)
                wl_sb[3] = wpool.tile([128, NCH[1] * 512], BF16, tag="wl",
                                      bufs=2, name="wl3")
                nc.gpsimd.dma_start(wl_sb[3][:, :NCH[3] * 512], wl[3][:])

                # ---- projection ctx-half, interleaved with r1/r2 -----------
                parts = [None] * NV

                def proj_ctx(vb):
                    ps = gps.tile([B, 512], F32, tag="ps512", bufs=2, name="psC")
                    nc.tensor.matmul(ps[:], onesr[:],
                                     bout_sb[:, vb * 512:(vb + 1) * 512],
                                     start=True, stop=False)
                    for k in range(8):
                        nc.tensor.matmul(ps[:], ctxT[:, k * B:(k + 1) * B],
                                         wC_sb[vb][:, k * 512:(k + 1) * 512],
                                         start=False, stop=(k == 7))
                    pt = acts.tile([B, 512], BF16, tag="parts", bufs=NV,
                                   name="parts")
                    nc.vector.tensor_copy(pt[:], ps[:])
                    parts[vb] = pt

                proj_ctx(0)
                load_wC(2)
                proj_ctx(1)
                load_wC(3)
                h3, h3T = lstm_layer(2, [(h2g, k * B) for k in range(8)]
                                     + hin_chunks(2))
                h3g = allgather(h3T, "h3")
                proj_ctx(2)
                load_wC(4)
                proj_ctx(3)
                load_wC(5)
                load_wH(3)
                h4, h4T = lstm_layer(3, [(h3g, k * B) for k in range(8)]
                                     + hin_chunks(3))
                h4g = allgather(h4T, "h4")
                proj_ctx(4)
                load_wC(6)
                proj_ctx(5)
                load_wC(7)
                load_wH(4)
                proj_ctx(6)
                proj_ctx(7)

            if STAGE == 3:
                hf = acts.tile([128, 8 * B], F32, tag="dbg", name="hf")
                nc.vector.tensor_copy(hf[:], h4g[:])
                for k in range(8):
                    nc.sync.dma_start(out[:, k * 128:(k + 1) * 128],
                                      hf[:, k * B:(k + 1) * B])

            if STAGE >= 4:
                # ---- projection h-half + combine + store -------------------
                for vb in range(NV):
                    ps = gps.tile([B, 512], F32, tag="ps512", bufs=2, name="psH")
                    for k in range(8):
                        nc.tensor.matmul(ps[:], h4g[:, k * B:(k + 1) * B],
                                         wH_sb[vb][:, k * 512:(k + 1) * 512],
                                         start=(k == 0), stop=(k == 7))
                    if vb < 3:
                        load_wH(vb + 5)
                    ot = acts.tile([B, 512], F32, tag="outsb", bufs=2,
                                   name="outsb")
                    nc.vector.tensor_tensor(ot[:], ps[:], parts[vb][:], ALU.add)
                    nc.sync.dma_start(out[:, vb * 512:(vb + 1) * 512], ot[:])

    nc.compile()
    return nc


def _prep_in_maps(inputs):
    f32 = lambda a: np.asarray(a, dtype=np.float32)
    bf = lambda a: np.ascontiguousarray(np.asarray(a, dtype=np.float32)
                                        .astype(NPBF))
    tokens = np.asarray(inputs["tokens"]).astype(np.int64)
    Emb = f32(inputs["E"])
    context = f32(inputs["context"])
    hidden = f32(inputs["hidden"])
    cell = f32(inputs["cell"])
    enc_out = f32(inputs["enc_outputs"])

    x = Emb[tokens]                                        # [B, 64]
    xc = np.concatenate([x, context], axis=1)              # [B, 1088]
    xc = np.pad(xc, ((0, 0), (0, XC - xc.shape[1])))       # [B, 1152]
    xcT = np.ascontiguousarray(xc.T).reshape(9, 128, B)    # [9, 128, B]
    xcT = bf(xcT.transpose(1, 0, 2).reshape(128, 9 * B))
    hT = hidden.transpose(0, 2, 1)                         # [NL, H, B]
    hTin = bf(hT.reshape(NL, 8, 128, B).transpose(2, 0, 1, 3)
              .reshape(128, NL * 8 * B))

    wih_full = [f32(inputs["W_ih_f"]), f32(inputs["W_ih_l0"]),
                f32(inputs["W_ih_rest"])[0], f32(inputs["W_ih_rest"])[1]]
    whh_full = [f32(inputs["W_hh_f"]), f32(inputs["W_hh_l0"]),
                f32(inputs["W_hh_rest"])[0], f32(inputs["W_hh_rest"])[1]]
    b_full = [f32(inputs["b_ih_f"]) + f32(inputs["b_hh_f"]),
              f32(inputs["b_ih_l0"]) + f32(inputs["b_hh_l0"]),
              f32(inputs["b_ih_rest"])[0] + f32(inputs["b_hh_rest"])[0],
              f32(inputs["b_ih_rest"])[1] + f32(inputs["b_hh_rest"])[1]]

    wadT = np.ascontiguousarray(f32(inputs["Wad"]).T)      # [H, 128]
    wadTp = bf(wadT.reshape(8, 128, KATT).transpose(1, 0, 2)
               .reshape(128, 8 * KATT))
    waen = bf(inputs["Wae"])                               # [128, E]
    baec = bf(f32(inputs["bae"]).reshape(KATT, 1))
    badc = np.ascontiguousarray(f32(inputs["bad"]).reshape(KATT, 1))
    Wout = f32(inputs["Wout"])
    bout_full = f32(inputs["bout"])
    identb = np.eye(128, dtype=NPBF)

    def gate_shard(W, c):
        # [4096, in] -> [in, 512]: rows for gates i,f,g,o of hidden dims
        # c*128:(c+1)*128, transposed.
        rows = np.concatenate(
            [W[g * H + c * HSH: g * H + (c + 1) * HSH] for g in range(4)], axis=0)
        return np.ascontiguousarray(rows.T)

    in_maps = []
    for c in range(NCORES):
        cn = cell[:, :, c * HSH:(c + 1) * HSH]             # [NL, B, 128]
        m = {"xcT": xcT, "hTin": hTin,
             "cnat": np.ascontiguousarray(cn.transpose(1, 0, 2).reshape(B, NL * HSH)),
             "wadT": wadTp, "badc": badc, "waen": waen, "baec": baec,
             "identb": identb,
             "encN": bf(enc_out[c * SSH:(c + 1) * SSH]
                        .transpose(1, 0, 2).reshape(B, SSH * E)),
             "boutr": np.ascontiguousarray(
                 np.pad(bout_full[c * VSH:(c + 1) * VSH],
                        (0, VPAD - VSH)).reshape(1, VPAD).astype(NPBF))}
        browp = np.zeros((1, NL * 512), np.float32)
        for l in range(NL):
            wt = gate_shard(wih_full[l], c)                # [in, 512]
            if l == 0:
                wt = np.pad(wt, ((0, XC - wt.shape[0]), (0, 0)))
            wcat = np.concatenate([wt, gate_shard(whh_full[l], c)], axis=0)
            nch = NCH[l]
            assert wcat.shape[0] == nch * 128
            m[f"wl{l}"] = bf(wcat.reshape(nch, 128, 512).transpose(1, 0, 2)
                             .reshape(128, nch * 512))
            b = b_full[l]
            browp[0, l * 512:(l + 1) * 512] = np.concatenate(
                [b[g * H + c * HSH: g * H + (c + 1) * HSH] for g in range(4)])
        m["brow"] = browp.astype(NPBF)
        Wsh = Wout[c * VSH:(c + 1) * VSH]                  # [4000, 2048]
        Wsh = np.pad(Wsh, ((0, VPAD - VSH), (0, 0)))       # [4096, 2048]
        WT = np.ascontiguousarray(Wsh.T)                   # [2048, 4096]
        m["woutH"] = bf(WT[0:1024].reshape(8, 128, NV, 512)
                        .transpose(1, 2, 0, 3).reshape(128, NV * 8 * 512))
        m["woutC"] = bf(WT[1024:2048].reshape(8, 128, NV, 512)
                        .transpose(1, 2, 0, 3).reshape(128, NV * 8 * 512))
        in_maps.append(m)
    return in_maps


def get_compiled():
    global _compiled
    if _compiled is None:
        _compiled = _build()
    return _compiled


def kernel(**inputs):
    nc = get_compiled()
    in_maps = _prep_in_maps(inputs)
    res = run_bass_kernel_spmd(nc, in_maps, core_ids=list(range(NCORES)))
    out = np.concatenate([res.results[c]["out"][:, :VSH] for c in range(NCORES)],
                         axis=1)
    return out


# revision 24
# speedup vs baseline: 1.5638x; 1.5638x over previous
"""Trainium2 Bass kernel for a 4-layer LSTM decoder step with Bahdanau attention.

v2: bf16 streaming, gates-on-free-dim LSTM, PE-based context accumulation,
coalesced [128, N] DMAs (bulk on SWDGE, latency-critical on SP HWDGE),
bf16 collectives, proj ctx-half interleaved into collective gaps.

Math (B=128 batch, S=128 enc positions, H=A=E_enc=1024, emb=64, V=32000, NL=4):
  x   = E[tokens]
  o1  = LSTM_f([x, context], hidden0, cell0)
  ad  = o1 @ Wad.T + bad ; w = ad @ Wae ; cdot = ad . bae
  scores[s,b] = enc[s,b,:].w[b,:] + cdot[b] ; alpha = softmax_s
  ctx = sum_s alpha[s,b] * enc[s,b,:]   (accumulated as diag(alpha_s) @ enc_s)
  h   = LSTM_l0([o1, ctx]) -> LSTM_r1(h) -> LSTM_r2(h)
  out = [h, ctx] @ Wout.T + bout                               # [128, 32000]

Distribution over 8 NeuronCores:
  - LSTM: gate rows sharded 8-way (each core computes a 128-wide hidden
    shard); full h re-assembled with a bf16 AllGather after every layer.
  - Attention: sharded over encoder positions s (16 per core); partial
    exp-weighted context + sum(exp) combined with one bf16 AllReduce.
  - Output projection: vocab-sharded (4000 rows of Wout per core, padded
    to 4096); shards concatenated on the host.
"""
import os
import sys

sys.path.insert(0, "/opt/trn_rl_repo")

STAGE = int(os.environ.get("KERNEL_STAGE", "4"))
NOCC = bool(int(os.environ.get("KERNEL_NOCC", "0")))  # timing-sim stand-in mode
CCF32 = bool(int(os.environ.get("KERNEL_CCF32", "0")))  # fp32 collectives fallback
TTR = bool(int(os.environ.get("KERNEL_TTR", "1")))  # fused tensor_tensor_reduce scores

import numpy as np
import ml_dtypes

from concourse import bacc, mybir, tile
from concourse.bass_utils import run_bass_kernel_spmd

F32 = mybir.dt.float32
BF16 = mybir.dt.bfloat16
ALU = mybir.AluOpType
ACT = mybir.ActivationFunctionType
NPBF = ml_dtypes.bfloat16

B = 128          # batch
S = 128          # encoder length
H = 1024         # hidden dim
NL = 4           # LSTM layers
KATT = 128       # attention projection size
E = 1024         # encoder hidden dim
NCORES = 8
HSH = H // NCORES        # 128: hidden shard per core
SSH = S // NCORES        # 16: encoder positions per core
VSH = 32000 // NCORES    # 4000: vocab shard
VPAD = 4096              # padded vocab shard (8 x 512)
NV = VPAD // 512         # 8 vocab blocks of 512
XC = 1152                # padded [emb(64) + context(1024)] input width (9 x 128)
NCH = (XC // 128 + 8, 2 * H // 128 + 8, H // 128 + 8, H // 128 + 8)  # 17,24,16,16
CCDT = F32 if CCF32 else BF16

_compiled = None


def _build():
    nc = bacc.Bacc("TRN2", target_bir_lowering=False, debug=False,
                   num_devices=NCORES)

    def din(name, shape, dt=BF16):
        return nc.dram_tensor(name, list(shape), dt, kind="ExternalInput").ap()

    xcT = din("xcT", [128, 9 * B])              # layer-f input chunks, transposed
    hTin = din("hTin", [128, NL * 8 * B])       # prev hidden chunks (l,k) transposed
    cnat = din("cnat", [128, NL * HSH], F32)    # cell shards, natural [B, l*128+j]
    wl = [din(f"wl{l}", [128, NCH[l] * 512]) for l in range(NL)]
    brow = din("brow", [1, NL * 512])      # gate bias rows per layer
    waen = din("waen", [KATT, E])               # Wae natural
    wadT = din("wadT", [128, 8 * KATT])         # Wad.T chunks
    baec = din("baec", [KATT, 1])
    badc = din("badc", [KATT, 1], F32)
    encN = din("encN", [B, SSH * E])            # enc natural [b, s*E+e]
    woutC = din("woutC", [128, NV * 8 * 512])   # ctx-half of WoutT, per (vb,kc)
    woutH = din("woutH", [128, NV * 8 * 512])   # h-half of WoutT
    boutr = din("boutr", [1, VPAD])
    identb = din("identb", [128, 128])
    out = nc.dram_tensor("out", [B, VPAD], F32, kind="ExternalOutput").ap()

    rg = [list(range(NCORES))]

    with tile.TileContext(nc) as tc:
        with tc.tile_pool(name="const", bufs=1) as const, \
             tc.tile_pool(name="wpool", bufs=1) as wpool, \
             tc.tile_pool(name="encp", bufs=1) as encp, \
             tc.tile_pool(name="woutp", bufs=1) as woutp, \
             tc.tile_pool(name="acts", bufs=1) as acts, \
             tc.tile_pool(name="gps", bufs=1, space="PSUM") as gps, \
             tc.tile_pool(name="attps", bufs=1, space="PSUM") as attps, \
             tc.tile_pool(name="bigps", bufs=1, space="PSUM") as bigps, \
             tc.tile_pool(name="trps", bufs=1, space="PSUM") as trps, \
             tc.tile_pool(name="dram", bufs=1, space="DRAM") as dram:

            # ---------------- constant / small loads (SP HWDGE ring) --------
            def cload(name, src, shape, dt):
                t = const.tile(list(shape), dt, tag=name, name=name)
                nc.sync.dma_start(t[:], src)
                return t

            ident_sb = cload("identb", identb[:], [128, 128], BF16)
            brow_sb = cload("brow", brow[:], [1, NL * 512], BF16)
            bout_sb = cload("boutr", boutr[:], [1, VPAD], BF16)
            waen_sb = cload("waen", waen[:], [KATT, E], BF16)
            wadT_sb = cload("wadT", wadT[:], [128, 8 * KATT], BF16)
            baec_sb = cload("baec", baec[:], [KATT, 1], BF16)
            badc_sb = cload("badc", badc[:], [KATT, 1], F32)
            onesr = const.tile([1, B], BF16, tag="onesr", name="onesr")
            nc.vector.memset(onesr[:], 1.0)

            # ---------------- bulk loads (SWDGE / gpsimd ring) --------------
            # Issue order == arrival order on the ring: early-needed first.
            # Interleaved with the collectives (also on gpsimd) so
            # buffer-reuse waits cannot deadlock against them.
            def bload(name, src, shape, dt=BF16, pool=None, tag=None, bufs=None):
                t = (pool or const).tile(list(shape), dt, tag=tag or name,
                                         bufs=bufs, name=name)
                nc.gpsimd.dma_start(t[:], src)
                return t

            WHB = 5 if CCF32 else 6
            xcT_sb = bload("xcT", xcT[:], [128, 9 * B])
            wl_sb = [None] * NL
            for l in (0, 1):
                wl_sb[l] = wpool.tile([128, NCH[1] * 512], BF16, tag="wl",
                                      bufs=2, name=f"wl{l}")
                if l == 1:
                    # keep need-order on the ring: hTin/cnat land between
                    # the two weight streams
                    hTin_sb = woutp.tile([128, NL * 8 * B], BF16, tag="wH",
                                         bufs=WHB, name="hTin")
                    nc.gpsimd.dma_start(hTin_sb[:], hTin[:])
                    cnat_sb = bload("cnat", cnat[:], [128, NL * HSH], F32)
                nc.gpsimd.dma_start(wl_sb[l][:, :NCH[l] * 512], wl[l][:])
            encN_sb = encp.tile([B, SSH * E], BF16, tag="encN", name="encN_sb")
            half = SSH * E // 2
            nc.gpsimd.dma_start(encN_sb[:, 0:half], encN[:, 0:half])
            nc.gpsimd.dma_start(encN_sb[:, half:], encN[:, half:])
            wC_sb = [None] * NV
            wH_sb = [None] * NV

            def load_wC(vb):
                wC_sb[vb] = woutp.tile([128, 8 * 512], BF16, tag="wC", bufs=2,
                                       name="wC")
                nc.gpsimd.dma_start(wC_sb[vb][:],
                                    woutC[:, vb * 4096:(vb + 1) * 4096])

            def load_wH(vb):
                wH_sb[vb] = woutp.tile([128, 8 * 512], BF16, tag="wH",
                                       bufs=WHB, name="wH")
                nc.gpsimd.dma_start(wH_sb[vb][:],
                                    woutH[:, vb * 4096:(vb + 1) * 4096])

            # ---------------- helpers ---------------------------------------
            def allgather(src_sb, name):
                """src_sb [128, B] bf16 -> [128, 8*B] sbuf tile of full hT."""
                cc_in = dram.tile([HSH, B], CCDT, tag=f"agi_{name}")
                cc_out = dram.tile([H, B], CCDT, tag=f"ago_{name}")
                nc.gpsimd.dma_start(cc_in[:], src_sb[:])
                if NOCC:
                    nc.gpsimd.dma_start(cc_out[0:HSH, :], cc_in[:])
                else:
                    nc.gpsimd.collective_compute(
                        "AllGather", ALU.bypass, replica_groups=rg,
                        ins=[cc_in[:].opt()], outs=[cc_out[:].opt()])
                t = acts.tile([128, 8 * B], BF16, tag="agh", bufs=3, name="agh")
                # partition p <- rows {p, 128+p, ...}: 8 segments per partition
                src_ap = cc_out[:].rearrange("(k p) b -> p k b", p=128)
                dst_ap = t[:].rearrange("p (k b) -> p k b", k=8)
                nc.gpsimd.dma_start(dst_ap, src_ap)
                return t

            def transpose_h(h_nat, tag="htr"):
                """h_nat [B, 128] bf16 -> [128, B] bf16 sbuf tile."""
                tp = trps.tile([128, B], BF16, tag="trp", bufs=2, name="trp")
                nc.tensor.transpose(tp[:], h_nat[:], ident_sb[:])
                t = acts.tile([128, B], BF16, tag=tag, bufs=2, name=tag)
                nc.scalar.activation(t[:], tp[:], ACT.Copy)
                return t

            def lstm_layer(l, in_chunks):
                """in_chunks: list of ([128, B] tile, free-offset) bf16 chunk
                sources for the layer input (transposed). Issues the gate
                matmuls; returns (h_nat, hT) tiles."""
                gps_t = gps.tile([B, 512], F32, tag="ps512", bufs=2,
                                 name=f"gates{l}")
                nc.tensor.matmul(gps_t[:], onesr[:],
                                 brow_sb[:, l * 512:(l + 1) * 512],
                                 start=True, stop=False)
                nch = NCH[l]
                for i, (src, off) in enumerate(in_chunks):
                    nc.tensor.matmul(gps_t[:], src[:, off:off + B],
                                     wl_sb[l][:, i * 512:(i + 1) * 512],
                                     start=False, stop=(i == nch - 1))
                assert len(in_chunks) == nch
                ifgo = []
                for g, fn in enumerate((ACT.Sigmoid, ACT.Sigmoid, ACT.Tanh,
                                        ACT.Sigmoid)):
                    t = acts.tile([B, 128], F32, tag="ifgo", bufs=8, name="ifgo")
                    nc.scalar.activation(t[:], gps_t[:, g * 128:(g + 1) * 128], fn)
                    ifgo.append(t)
                t1 = acts.tile([B, 128], F32, tag="ctmp", bufs=6, name="ctmp")
                nc.vector.tensor_tensor(t1[:], ifgo[1][:],
                                        cnat_sb[:, l * HSH:(l + 1) * HSH],
                                        ALU.mult)
                t2 = acts.tile([B, 128], F32, tag="ctmp", bufs=6, name="ctmp")
                nc.vector.tensor_tensor(t2[:], ifgo[0][:], ifgo[2][:], ALU.mult)
                c2 = acts.tile([B, 128], F32, tag="ctmp", bufs=6, name="ctmp")
                nc.vector.tensor_tensor(c2[:], t1[:], t2[:], ALU.add)
                tc2 = acts.tile([B, 128], F32, tag="ctmp", bufs=6, name="ctmp")
                nc.scalar.activation(tc2[:], c2[:], ACT.Tanh)
                h = acts.tile([B, 128], BF16, tag="hnat", bufs=2, name="hnat")
                nc.vector.tensor_tensor(h[:], ifgo[3][:], tc2[:], ALU.mult)
                return h, transpose_h(h)

            def hin_chunks(l):
                return [(hTin_sb, (l * 8 + k) * B) for k in range(8)]

            # ---------------- layer f ---------------------------------------
            h1, h1T = lstm_layer(0, [(xcT_sb, k * B) for k in range(9)]
                                 + hin_chunks(0))
            o1T = allgather(h1T, "h1")
            # bulk wave 2 (post-AG1 on the gpsimd ring): layer-2 weights and
            # the first output-projection tiles
            wl_sb[2] = wpool.tile([128, NCH[1] * 512], BF16, tag="wl",
                                  bufs=2, name="wl2")
            nc.gpsimd.dma_start(wl_sb[2][:, :NCH[2] * 512], wl[2][:])
            load_wC(0)
            load_wC(1)
            load_wH(0)
            load_wH(1)
            load_wH(2)
            if STAGE == 1:
                o1f = acts.tile([128, 8 * B], F32, tag="dbg", name="o1f")
                nc.vector.tensor_copy(o1f[:], o1T[:])
                for k in range(8):
                    nc.sync.dma_start(out[:, k * 128:(k + 1) * 128],
                                      o1f[:, k * B:(k + 1) * B])

            if STAGE >= 2:
                # ---------------- attention ---------------------------------
                # adT[k, b] = Wad @ o1 + bad
                ad_ps = attps.tile([KATT, B], F32, tag="att128", bufs=2,
                                   name="ad_ps")
                for k in range(8):
                    nc.tensor.matmul(ad_ps[:], wadT_sb[:, k * KATT:(k + 1) * KATT],
                                     o1T[:, k * B:(k + 1) * B],
                                     start=(k == 0), stop=(k == 7))
                adT_sb = acts.tile([KATT, B], BF16, tag="adT", name="adT_sb")
                nc.scalar.activation(adT_sb[:], ad_ps[:], ACT.Identity,
                                     bias=badc_sb[:])
                # cdot[b] = ad . bae
                cd_ps = attps.tile([B, 1], F32, tag="att128", bufs=2, name="cd_ps")
                nc.tensor.matmul(cd_ps[:], adT_sb[:], baec_sb[:],
                                 start=True, stop=True)
                cdot_sb = acts.tile([B, 1], F32, tag="cdot", name="cdot_sb")
                nc.vector.tensor_copy(cdot_sb[:], cd_ps[:])
                # w[b, e] = ad @ Wae
                w_ps = bigps.tile([B, E], F32, tag="ps1024", bufs=1, name="w_ps")
                for half in range(2):
                    nc.tensor.matmul(w_ps[:, half * 512:(half + 1) * 512],
                                     adT_sb[:],
                                     waen_sb[:, half * 512:(half + 1) * 512],
                                     start=True, stop=True)
                w_sb = acts.tile([B, E], BF16, tag="w_sb", name="w_sb")
                nc.scalar.activation(w_sb[:], w_ps[:], ACT.Copy)
                # scores -> alpha columns; ctx accumulated via diag trick
                alpha = acts.tile([B, SSH], F32, tag="alpha", name="alpha")
                ctx_ps = bigps.tile([B, E], F32, tag="ps1024", bufs=1,
                                    name="ctx_ps")
                prodd = acts.tile([B, E], BF16, tag="prodd", bufs=1, name="prodd")
                for s in range(SSH):
                    sc = acts.tile([B, 1], F32, tag="sccol", bufs=2, name="sccol")
                    if TTR:
                        nc.vector.tensor_tensor_reduce(
                            prodd[:], encN_sb[:, s * E:(s + 1) * E], w_sb[:],
                            1.0, 0.0, ALU.mult, ALU.add, accum_out=sc[:])
                    else:
                        nc.vector.tensor_tensor(
                            prodd[:], encN_sb[:, s * E:(s + 1) * E], w_sb[:],
                            ALU.mult)
                        nc.vector.tensor_reduce(sc[:], prodd[:],
                                                mybir.AxisListType.X, ALU.add)
                    nc.scalar.activation(alpha[:, s:s + 1], sc[:], ACT.Exp,
                                         bias=cdot_sb[:])
                    dg = acts.tile([B, B], BF16, tag="diag", bufs=2, name="diag")
                    nc.vector.tensor_tensor(dg[:], ident_sb[:],
                                            alpha[:, s:s + 1].to_broadcast([B, B]),
                                            ALU.mult)
                    for half in range(2):
                        nc.tensor.matmul(ctx_ps[:, half * 512:(half + 1) * 512],
                                         dg[:],
                                         encN_sb[:, s * E + half * 512:
                                                 s * E + (half + 1) * 512],
                                         start=(s == 0), stop=(s == SSH - 1))
                sume = acts.tile([B, 1], F32, tag="sume", name="sume")
                nc.vector.tensor_reduce(sume[:], alpha[:], mybir.AxisListType.X,
                                        ALU.add)
                # AllReduce partial [ctxU | sumexp]
                ccpad = 16 if CCDT == BF16 else 8
                ar_in = dram.tile([B, E + ccpad], CCDT, tag="ar_in")
                ar_out = dram.tile([B, E + ccpad], CCDT, tag="ar_out")
                ctxu_sb = acts.tile([B, E + ccpad], CCDT, tag="ctxu",
                                    name="ctxu_sb")
                nc.vector.tensor_copy(ctxu_sb[:, 0:E], ctx_ps[:])
                nc.vector.tensor_copy(ctxu_sb[:, E:E + ccpad],
                                      sume[:].to_broadcast([B, ccpad]))
                nc.gpsimd.dma_start(ar_in[:], ctxu_sb[:])
                if NOCC:
                    nc.gpsimd.dma_start(ar_out[:], ar_in[:])
                else:
                    nc.gpsimd.collective_compute(
                        "AllReduce", ALU.add, replica_groups=rg,
                        ins=[ar_in[:].opt()], outs=[ar_out[:].opt()])
                ctxg = acts.tile([B, E + ccpad], CCDT, tag="ctxg", name="ctxg")
                nc.gpsimd.dma_start(ctxg[:], ar_out[:])
                seg = acts.tile([B, 1], F32, tag="seg", name="seg")
                nc.vector.tensor_copy(seg[:], ctxg[:, E:E + 1])
                recip = acts.tile([B, 1], F32, tag="recip", name="recip")
                nc.vector.reciprocal(recip[:], seg[:])
                ctx_sb = acts.tile([B, E], BF16, tag="ctx_sb", name="ctx_sb")
                nc.scalar.activation(ctx_sb[:], ctxg[:, 0:E], ACT.Copy,
                                     scale=recip[:])

            if STAGE == 2:
                cf = acts.tile([B, E], F32, tag="dbg", name="cf")
                nc.vector.tensor_copy(cf[:], ctx_sb[:])
                nc.sync.dma_start(out[:, 0:E], cf[:])

            if STAGE >= 3:
                # ---------------- layer l0 ----------------------------------
                # o1-half of the gates can start before ctx is ready.
                gps_l0 = gps.tile([B, 512], F32, tag="ps512", bufs=2,
                                  name="gates1")
                nc.tensor.matmul(gps_l0[:], onesr[:], brow_sb[:, 512:1024],
                                 start=True, stop=False)
                for k in range(8):
                    nc.tensor.matmul(gps_l0[:], o1T[:, k * B:(k + 1) * B],
                                     wl_sb[1][:, k * 512:(k + 1) * 512],
                                     start=False, stop=False)
                for k in range(8):
                    nc.tensor.matmul(gps_l0[:], hTin_sb[:, (8 + k) * B:(9 + k) * B],
                                     wl_sb[1][:, (16 + k) * 512:(17 + k) * 512],
                                     start=False, stop=False)
                # ctx transposed chunks (also feed the output projection)
                ctxT = acts.tile([128, 8 * B], BF16, tag="ctxT", name="ctxT")
                for k in range(8):
                    tp = trps.tile([128, B], BF16, tag="trp", bufs=2, name="trp")
                    nc.tensor.transpose(tp[:], ctx_sb[:, k * 128:(k + 1) * 128],
                                        ident_sb[:])
                    nc.scalar.activation(ctxT[:, k * B:(k + 1) * B], tp[:],
                                         ACT.Copy)
                for k in range(8):
                    nc.tensor.matmul(gps_l0[:], ctxT[:, k * B:(k + 1) * B],
                                     wl_sb[1][:, (8 + k) * 512:(9 + k) * 512],
                                     start=False, stop=(k == 7))
                ifgo = []
                for g, fn in enumerate((ACT.Sigmoid, ACT.Sigmoid, ACT.Tanh,
                                        ACT.Sigmoid)):
                    t = acts.tile([B, 128], F32, tag="ifgo", bufs=8, name="ifgo")
                    nc.scalar.activation(t[:], gps_l0[:, g * 128:(g + 1) * 128],
                                         fn)
                    ifgo.append(t)
                t1 = acts.tile([B, 128], F32, tag="ctmp", bufs=6, name="ctmp")
                nc.vector.tensor_tensor(t1[:], ifgo[1][:],
                                        cnat_sb[:, HSH:2 * HSH], ALU.mult)
                t2 = acts.tile([B, 128], F32, tag="ctmp", bufs=6, name="ctmp")
                nc.vector.tensor_tensor(t2[:], ifgo[0][:], ifgo[2][:], ALU.mult)
                c2 = acts.tile([B, 128], F32, tag="ctmp", bufs=6, name="ctmp")
                nc.vector.tensor_tensor(c2[:], t1[:], t2[:], ALU.add)
                tc2 = acts.tile([B, 128], F32, tag="ctmp", bufs=6, name="ctmp")
                nc.scalar.activation(tc2[:], c2[:], ACT.Tanh)
                h2 = acts.tile([B, 128], BF16, tag="hnat", bufs=2, name="hnat")
                nc.vector.tensor_tensor(h2[:], ifgo[3][:], tc2[:], ALU.mult)
                h2T = transpose_h(h2)
                h2g = allgather(h2T, "h2")
                # bulk: layer-3 weights (reuses wl1's slot; l0 mms are done)
                wl_sb[3] = wpool.tile([128, NCH[1] * 512], BF16, tag="wl",
                                      bufs=2, name="wl3")
                nc.gpsimd.dma_start(wl_sb[3][:, :NCH[3] * 512], wl[3][:])

                # ---- projection ctx-half, interleaved with r1/r2 -----------
                parts = [None] * NV

                def proj_ctx(vb):
                    ps = gps.tile([B, 512], F32, tag="ps512", bufs=2, name="psC")
                    nc.tensor.matmul(ps[:], onesr[:],
                                     bout_sb[:, vb * 512:(vb + 1) * 512],
                                     start=True, stop=False)
                    for k in range(8):
                        nc.tensor.matmul(ps[:], ctxT[:, k * B:(k + 1) * B],
                                         wC_sb[vb][:, k * 512:(k + 1) * 512],
                                         start=False, stop=(k == 7))
                    pt = acts.tile([B, 512], BF16, tag="parts", bufs=NV,
                                   name="parts")
                    nc.vector.tensor_copy(pt[:], ps[:])
                    parts[vb] = pt

                proj_ctx(0)
                load_wC(2)
                proj_ctx(1)
                load_wC(3)
                h3, h3T = lstm_layer(2, [(h2g, k * B) for k in range(8)]
                                     + hin_chunks(2))
                h3g = allgather(h3T, "h3")
                proj_ctx(2)
                load_wC(4)
                proj_ctx(3)
                load_wC(5)
                load_wH(3)
                h4, h4T = lstm_layer(3, [(h3g, k * B) for k in range(8)]
                                     + hin_chunks(3))
                h4g = allgather(h4T, "h4")
                proj_ctx(4)
                load_wC(6)
                proj_ctx(5)
                load_wC(7)
                load_wH(4)
                proj_ctx(6)
                proj_ctx(7)

            if STAGE == 3:
                hf = acts.tile([128, 8 * B], F32, tag="dbg", name="hf")
                nc.vector.tensor_copy(hf[:], h4g[:])
                for k in range(8):
                    nc.sync.dma_start(out[:, k * 128:(k + 1) * 128],
                                      hf[:, k * B:(k + 1) * B])

            if STAGE >= 4:
                # ---- projection h-half + combine + store -------------------
                for vb in range(NV):
                    ps = gps.tile([B, 512], F32, tag="ps512", bufs=2, name="psH")
                    for k in range(8):
                        nc.tensor.matmul(ps[:], h4g[:, k * B:(k + 1) * B],
                                         wH_sb[vb][:, k * 512:(k + 1) * 512],
                                         start=(k == 0), stop=(k == 7))
                    if vb < 3:
                        load_wH(vb + 5)
                    ot = acts.tile([B, 512], F32, tag="outsb", bufs=2,
                                   name="outsb")
                    nc.vector.tensor_tensor(ot[:], ps[:], parts[vb][:], ALU.add)
                    nc.sync.dma_start(out[:, vb * 512:(vb + 1) * 512], ot[:])

    nc.compile()
    return nc


def _prep_in_maps(inputs):
    f32 = lambda a: np.asarray(a, dtype=np.float32)
    bf = lambda a: np.ascontiguousarray(np.asarray(a, dtype=np.float32)
                                        .astype(NPBF))
    tokens = np.asarray(inputs["tokens"]).astype(np.int64)
    Emb = f32(inputs["E"])
    context = f32(inputs["context"])
    hidden = f32(inputs["hidden"])
    cell = f32(inputs["cell"])
    enc_out = f32(inputs["enc_outputs"])

    x = Emb[tokens]                                        # [B, 64]
    xc = np.concatenate([x, context], axis=1)              # [B, 1088]
    xc = np.pad(xc, ((0, 0), (0, XC - xc.shape[1])))       # [B, 1152]
    xcT = np.ascontiguousarray(xc.T).reshape(9, 128, B)    # [9, 128, B]
    xcT = bf(xcT.transpose(1, 0, 2).reshape(128, 9 * B))
    hT = hidden.transpose(0, 2, 1)                         # [NL, H, B]
    hTin = bf(hT.reshape(NL, 8, 128, B).transpose(2, 0, 1, 3)
              .reshape(128, NL * 8 * B))

    wih_full = [f32(inputs["W_ih_f"]), f32(inputs["W_ih_l0"]),
                f32(inputs["W_ih_rest"])[0], f32(inputs["W_ih_rest"])[1]]
    whh_full = [f32(inputs["W_hh_f"]), f32(inputs["W_hh_l0"]),
                f32(inputs["W_hh_rest"])[0], f32(inputs["W_hh_rest"])[1]]
    b_full = [f32(inputs["b_ih_f"]) + f32(inputs["b_hh_f"]),
              f32(inputs["b_ih_l0"]) + f32(inputs["b_hh_l0"]),
              f32(inputs["b_ih_rest"])[0] + f32(inputs["b_hh_rest"])[0],
              f32(inputs["b_ih_rest"])[1] + f32(inputs["b_hh_rest"])[1]]

    wadT = np.ascontiguousarray(f32(inputs["Wad"]).T)      # [H, 128]
    wadTp = bf(wadT.reshape(8, 128, KATT).transpose(1, 0, 2)
               .reshape(128, 8 * KATT))
    waen = bf(inputs["Wae"])                               # [128, E]
    baec = bf(f32(inputs["bae"]).reshape(KATT, 1))
    badc = np.ascontiguousarray(f32(inputs["bad"]).reshape(KATT, 1))
    Wout = f32(inputs["Wout"])
    bout_full = f32(inputs["bout"])
    identb = np.eye(128, dtype=NPBF)

    def gate_shard(W, c):
        # [4096, in] -> [in, 512]: rows for gates i,f,g,o of hidden dims
        # c*128:(c+1)*128, transposed.
        rows = np.concatenate(
            [W[g * H + c * HSH: g * H + (c + 1) * HSH] for g in range(4)], axis=0)
        return np.ascontiguousarray(rows.T)

    in_maps = []
    for c in range(NCORES):
        cn = cell[:, :, c * HSH:(c + 1) * HSH]             # [NL, B, 128]
        m = {"xcT": xcT, "hTin": hTin,
             "cnat": np.ascontiguousarray(cn.transpose(1, 0, 2).reshape(B, NL * HSH)),
             "wadT": wadTp, "badc": badc, "waen": waen, "baec": baec,
             "identb": identb,
             "encN": bf(enc_out[c * SSH:(c + 1) * SSH]
                        .transpose(1, 0, 2).reshape(B, SSH * E)),
             "boutr": np.ascontiguousarray(
                 np.pad(bout_full[c * VSH:(c + 1) * VSH],
                        (0, VPAD - VSH)).reshape(1, VPAD).astype(NPBF))}
        browp = np.zeros((1, NL * 512), np.float32)
        for l in range(NL):
            wt = gate_shard(wih_full[l], c)                # [in, 512]
            if l == 0:
                wt = np.pad(wt, ((0, XC - wt.shape[0]), (0, 0)))
            wcat = np.concatenate([wt, gate_shard(whh_full[l], c)], axis=0)
            nch = NCH[l]
            assert wcat.shape[0] == nch * 128
            m[f"wl{l}"] = bf(wcat.reshape(nch, 128, 512).transpose(1, 0, 2)
                             .reshape(128, nch * 512))
            b = b_full[l]
            browp[0, l * 512:(l + 1) * 512] = np.concatenate(
                [b[g * H + c * HSH: g * H + (c + 1) * HSH] for g in range(4)])
        m["brow"] = browp.astype(NPBF)
        Wsh = Wout[c * VSH:(c + 1) * VSH]                  # [4000, 2048]
        Wsh = np.pad(Wsh, ((0, VPAD - VSH), (0, 0)))       # [4096, 2048]
        WT = np.ascontiguousarray(Wsh.T)                   # [2048, 4096]
        m["woutH"] = bf(WT[0:1024].reshape(8, 128, NV, 512)
                        .transpose(1, 2, 0, 3).reshape(128, NV * 8 * 512))
        m["woutC"] = bf(WT[1024:2048].reshape(8, 128, NV, 512)
                        .transpose(1, 2, 0, 3).reshape(128, NV * 8 * 512))
        in_maps.append(m)
    return in_maps


def get_compiled():
    global _compiled
    if _compiled is None:
        _compiled = _build()
    return _compiled


def kernel(**inputs):
    nc = get_compiled()
    in_maps = _prep_in_maps(inputs)
    res = run_bass_kernel_spmd(nc, in_maps, core_ids=list(range(NCORES)))
    out = np.concatenate([res.results[c]["out"][:, :VSH] for c in range(NCORES)],
                         axis=1)
    return out
